# revision 1
# baseline (speedup 1.0000x reference)
"""NextVLAD + MPNCOV kernel for Trainium2 (8 NeuronCores, data-parallel over batch).

Strategy:
- Host: clip-of-8 regroup + L2 norm (cheap, memory-layout work), weight folding:
  the gk/g logits are linear in x1 = xn @ W_inp.T + b_inp, so
  logits_gk = xn @ (W_gk @ W_inp).T + (W_gk @ b_inp + b_gk)  -- halves device FLOPs.
- Device (8 cores, 1 sample each): the dominant matmul
  y_s = xn_s.T @ [W_inp.T | Wgk_fold.T | Wg_fold.T]  ([1568, 2310] per sample),
  K=768 contraction tiled 6x128, fp32r matmuls (full PE rate, N=462>=256).
- Host epilogue: sigmoid, softmax over tokens, VLAD aggregation, W_red,
  covariance pooling + Newton-Schulz sqrt (tiny 48x48 mats), upper-tri extract.

If the device path fails for any reason, a numpy fallback produces the same
result (kernel stays correct, just not accelerated).
"""

import sys
import numpy as np

for _p in ("/opt/trn_rl_repo",):
    if _p not in sys.path:
        sys.path.insert(0, _p)

BS8, C, H, W = 64, 768, 14, 14
GROUPS, K, EXP, OUT = 6, 128, 2, 48
D = EXP * C // GROUPS  # 256
BS = BS8 // 8          # 8 samples
M = 8 * H * W          # 1568 tokens per sample
N2 = EXP * C           # 1536
NCAT = N2 + GROUPS * K + GROUPS  # 2310
N_CORES = 8

_NC_CACHE = {}


def _build_nc():
    import concourse.bass as bass
    import concourse.tile as tile
    from concourse import mybir

    nc = bass.Bass()
    xt = nc.dram_tensor("xt", [C, M], mybir.dt.float32, kind="ExternalInput")
    wc = nc.dram_tensor("wc", [C, NCAT], mybir.dt.float32, kind="ExternalInput")
    y = nc.dram_tensor("y", [M, NCAT], mybir.dt.float32, kind="ExternalOutput")

    KT = C // 128          # 6 contraction tiles
    MT = (M + 127) // 128  # 13 token tiles (last = 32)
    NTS = 462              # 2310 / 5, >=256 keeps fp32r at full rate
    NT = NCAT // NTS       # 5

    xt_r = xt[:, :].rearrange("(k p) m -> k p m", p=128)
    wc_r = wc[:, :].rearrange("(k p) n -> k p n", p=128)

    with tile.TileContext(nc) as tc:
        with (
            tc.tile_pool(name="xp", bufs=1) as xp,
            tc.tile_pool(name="wp", bufs=1) as wp,
            tc.tile_pool(name="ps", bufs=8, space="PSUM") as ps,
            tc.tile_pool(name="ob", bufs=8) as ob,
        ):
            xsb = xp.tile([128, KT, M], mybir.dt.float32)
            wsb = wp.tile([128, KT, NCAT], mybir.dt.float32)
            for k in range(KT):
                nc.sync.dma_start(out=xsb[:, k, :], in_=xt_r[k])
                nc.sync.dma_start(out=wsb[:, k, :], in_=wc_r[k])

            for mt in range(MT):
                m0 = mt * 128
                msz = min(128, M - m0)
                for nt in range(NT):
                    n0 = nt * NTS
                    pt = ps.tile([128, NTS], mybir.dt.float32)
                    for k in range(KT):
                        lhsT = xsb[:, k, m0 : m0 + msz].bitcast(mybir.dt.float32r)
                        rhs = wsb[:, k, n0 : n0 + NTS].bitcast(mybir.dt.float32r)
                        nc.tensor.matmul(
                            pt[:msz, :], lhsT, rhs,
                            start=(k == 0), stop=(k == KT - 1),
                        )
                    ot = ob.tile([128, NTS], mybir.dt.float32)
                    nc.vector.tensor_copy(out=ot[:msz, :], in_=pt[:msz, :])
                    nc.sync.dma_start(
                        out=y[m0 : m0 + msz, n0 : n0 + NTS], in_=ot[:msz, :]
                    )
    return nc


def _run_device(xn, wcat):
    """xn: [BS, C, M] L2-normalized tokens (C-major); wcat: [C, NCAT].
    Returns [BS, M, NCAT] = xn[b].T @ wcat per sample, via 8 NeuronCores."""
    from concourse.bass_utils import run_bass_kernel_spmd

    if "nc" not in _NC_CACHE:
        _NC_CACHE["nc"] = _build_nc()
    nc = _NC_CACHE["nc"]
    wcat = np.ascontiguousarray(wcat, dtype=np.float32)
    in_maps = [
        {"xt": np.ascontiguousarray(xn[b], dtype=np.float32), "wc": wcat}
        for b in range(BS)
    ]
    res = run_bass_kernel_spmd(nc, in_maps, list(range(N_CORES))).results
    return np.stack([res[b]["y"] for b in range(BS)])


def _sqrtm_ns3(A):
    d = A.shape[-1]
    I3 = 3.0 * np.eye(d, dtype=np.float32)
    trA = np.trace(A, axis1=-2, axis2=-1)[..., None, None]
    An = A / trA
    ZY0 = 0.5 * (I3 - An)
    Y0 = An @ ZY0
    Z0 = ZY0
    ZY1 = 0.5 * (I3 - Z0 @ Y0)
    Y1 = Y0 @ ZY1
    Z1 = ZY1 @ Z0
    Yf = 0.5 * (Y1 @ (I3 - Z1 @ Y1))
    return Yf * np.sqrt(trA)


def kernel(x, centroids, W_inp, b_inp, W_g, b_g, W_gk, b_gk, W_red, b_red):
    x = np.asarray(x, dtype=np.float32)
    # clip-of-8 regroup: [64,C,H,W] -> [bs, C, M] (C-major == device lhsT layout)
    xr = (
        x.reshape(BS, 8, C, H, W)
        .transpose(0, 2, 1, 3, 4)
        .reshape(BS, C, M)
    )
    nrm = np.sqrt((xr.astype(np.float64) ** 2).sum(axis=1, keepdims=True))
    xn = (xr / np.maximum(nrm, 1e-12)).astype(np.float32)

    # fold the gk/g projections through W_inp (exact up to fp rounding)
    W_inp = np.asarray(W_inp, np.float32)
    Wgk_f = np.asarray(W_gk, np.float32) @ W_inp          # [768, 768]
    bgk_f = np.asarray(W_gk, np.float32) @ np.asarray(b_inp, np.float32) + b_gk
    Wg_f = np.asarray(W_g, np.float32) @ W_inp            # [6, 768]
    bg_f = np.asarray(W_g, np.float32) @ np.asarray(b_inp, np.float32) + b_g
    wcat = np.concatenate([W_inp.T, Wgk_f.T, Wg_f.T], axis=1)  # [768, 2310]
    bcat = np.concatenate([b_inp, bgk_f, bg_f]).astype(np.float32)

    try:
        y = _run_device(xn, wcat)
    except Exception as e:  # fallback: same math on host
        sys.stderr.write(f"[kernel.py] device path failed ({e!r}); numpy fallback\n")
        y = np.einsum("bcm,cn->bmn", xn, wcat, optimize=True)

    y = y + bcat  # [BS, M, NCAT]
    x1 = y[:, :, :N2]                      # [bs, M, 1536]
    lg_gk = y[:, :, N2 : N2 + GROUPS * K]  # [bs, M, 768]
    lg_g = y[:, :, N2 + GROUPS * K :]      # [bs, M, 6]

    alpha_g = 1.0 / (1.0 + np.exp(-lg_g))  # sigmoid
    t = lg_gk - lg_gk.max(axis=1, keepdims=True)  # softmax over tokens (dim=1)
    e = np.exp(t)
    a_gk = e / e.sum(axis=1, keepdims=True)
    a_gk = a_gk.reshape(BS, M, GROUPS, K)

    w = a_gk * alpha_g[..., None]          # [bs, M, G, K]
    xg = x1.reshape(BS, M, GROUPS, D)
    vlad = np.einsum("bmgk,bmgd->bgkd", w, xg, optimize=True)
    vlad = vlad - w.sum(axis=1)[..., None] * np.asarray(centroids, np.float32)
    vlad = vlad @ np.asarray(W_red, np.float32).T + b_red  # [bs, G, K, OUT]

    v = vlad.transpose(0, 3, 2, 1)                         # [bs, OUT, K, G]
    vk = v.transpose(0, 2, 1, 3).reshape(BS, K, OUT, GROUPS)
    I_hat = (np.eye(GROUPS, dtype=np.float32) / GROUPS) - 1.0 / (GROUPS * GROUPS)
    cov = vk @ I_hat @ vk.transpose(0, 1, 3, 2)            # [bs, K, 48, 48]
    sq = _sqrtm_ns3(cov.astype(np.float32))

    r, c = np.triu_indices(OUT)
    lin = r * OUT + c
    tri = sq.reshape(BS, K, OUT * OUT)[..., lin]
    return np.ascontiguousarray(tri.reshape(BS, K * tri.shape[-1]).astype(np.float32))
